# revision 27
# baseline (speedup 1.0000x reference)
"""Trainium2 Bass kernel for nn_LocalizedLoraLayer.

Math (full problem):
  out = x @ W.T + b + (alpha/r_block) * delta
  delta[:, :, j*bs:(j+1)*bs] = sum_k  (x_k @ A[k,j].T) @ B[k,j].T
  with x: [4, 2048, 4096], W: [4096, 4096] ([out, in]), A: [8, 8, 16, 512],
  B: [8, 8, 512, 16].

Strategy: data-parallel over tokens (8192 tokens -> 1024/core on 8 cores).
All matmul operands in bf16 (inputs quantized on host; rel err ~4e-3 vs the
2e-2 gate). Output produced transposed [d, tok] in bf16; host untransposes
and upcasts.

Per-core device schedule:
  x resident in SBUF as xt[128(i-local), ich*1024 + t] (8 MiB bf16),
  loaded in 1 MiB pieces split across both HWDGE rings. W streamed once as
  32x 1MiB slabs wsb[128(i-local), ich*128 + o-local].
  Warm-up junk matmuls flip the HAM clock gate to 8/8 during the x load.
  stage 1 (LoRA down-proj): per k_in block, T_k[(j,r), t] accumulated over
    4 i-chunks; evacuated to bf16, parked in DRAM scratch, and read back
    per output block j as tt[(k,r)=128, j*1024 + t] via a strided AP
    (the (k,r,t) source order matches the flat partition order).
  dense: per o-chunk oc (128 outs), psum [128 o, 512 t] x2 token halves
    accumulates 32 dense matmuls (W slab stationary, x moving) plus 1 LoRA
    matmul (bcat stationary, tt moving) = whole layer fused. Each stationary
    feeds both token-half matmuls; the second skips its weight load.
  bias b is added on host during unshard (b is zeros by spec).
"""

import numpy as np
import ml_dtypes

import concourse.bass as bass
import concourse.mybir as mybir
import concourse.tile as tile
from concourse import bacc
from concourse.bass_utils import run_bass_kernel_spmd

N_CORES = 8
TOK = 1024          # tokens per core
D = 4096            # model dim
KB = 8              # number of blocks (K)
BS = 512            # block size
R = 16              # lora rank
NIC = D // 128      # 32 i-chunks
NOC = D // 128      # 32 o-chunks of 128

F32 = mybir.dt.float32
BF16 = mybir.dt.bfloat16
NPBF16 = ml_dtypes.bfloat16

_CACHE = {}


def _build():
    nc = bacc.Bacc(None, target_bir_lowering=False)

    xt = nc.dram_tensor("xt", [128, NIC * TOK], BF16, kind="ExternalInput")
    wtr = nc.dram_tensor("wtr", [D, D], BF16, kind="ExternalInput")
    acat = nc.dram_tensor("acat", [128, D], BF16, kind="ExternalInput")
    bcat = nc.dram_tensor("bcat", [128, D], BF16, kind="ExternalInput")
    outr = nc.dram_tensor("outr", [D, TOK], BF16, kind="ExternalOutput")

    def mm_pair(p0, p1, lhsT, r0, r1, start, stop):
        # two matmuls sharing one stationary operand
        nc.tensor.matmul(p0[:], lhsT, r0, start=start, stop=stop)
        nc.tensor.matmul(p1[:], lhsT, r1, start=start, stop=stop)

    with tile.TileContext(nc) as tc:
        with (
            tc.tile_pool(name="res", bufs=1) as res,
            tc.tile_pool(name="wts", bufs=5) as wts,
            tc.tile_pool(name="evp", bufs=4) as evp,
            tc.tile_pool(name="osb", bufs=4) as osbp,
            tc.tile_pool(name="psd", bufs=1, space="PSUM") as psd,
            tc.tile_pool(name="dramp", bufs=1, space="DRAM") as dramp,
        ):
            acat_sb = res.tile([128, D], BF16)
            xt_sb = res.tile([128, NIC * TOK], BF16)
            bcat_sb = res.tile([128, D], BF16)
            tt_sb = res.tile([128, KB * TOK], BF16)
            scr_sb = res.tile([128, 512], BF16)
            evd = dramp.tile([KB, 128, TOK], BF16)

            wslab = {}

            def w_dma(oc):
                w_t = wts.tile([128, D], BF16, name=f"w{oc}", tag="w")
                nc.sync.dma_start(
                    w_t[:], wtr[oc * 128:(oc + 1) * 128, :]
                )
                wslab[oc] = w_t

            def x_dma(p):
                nc.sync.dma_start(
                    xt_sb[:, p * 4096:(p + 1) * 4096],
                    xt[:, p * 4096:(p + 1) * 4096],
                )

            def x_dma_s(p):
                nc.scalar.dma_start(
                    xt_sb[:, p * 4096:(p + 1) * 4096],
                    xt[:, p * 4096:(p + 1) * 4096],
                )

            # sync: acat head, x0a, x1, w0, x3, x5, x7, [ev/tt regroup], w2..
            # scalar: acat tail, x0b, x2, x4, x6, w1, bcat, [evacs/outs]
            nc.sync.dma_start(acat_sb[:, 0:1024], acat[:, 0:1024])
            nc.sync.dma_start(xt_sb[:, 0:2048], xt[:, 0:2048])
            nc.scalar.dma_start(acat_sb[:, 1024:4096], acat[:, 1024:4096])
            nc.scalar.dma_start(xt_sb[:, 2048:4096], xt[:, 2048:4096])
            x_dma(1)
            x_dma_s(2)
            w_dma(0)
            x_dma(3)
            x_dma_s(4)
            x_dma(5)
            x_dma_s(6)
            x_dma(7)
            w1_t = wts.tile([128, D], BF16, name="w1", tag="w")
            nc.scalar.dma_start(w1_t[:], wtr[128:256, :])
            wslab[1] = w1_t
            nc.scalar.dma_start(bcat_sb[:], bcat[:])

            # ---- PE warm-up: ~12 junk matmuls on scratch data flip the
            # HAM clock gate to 8/8 during the initial x load, so stage-1
            # runs at full clock. Output goes to a dead PSUM bank.
            nc.vector.memset(scr_sb[:], 0.0)
            ps_w = psd.tile([128, 512], F32, name="warm", tag="d10")
            for i in range(12):
                nc.tensor.matmul(
                    ps_w[:], scr_sb[:, 0:128], scr_sb[:],
                    start=(i == 0), stop=(i == 11),
                )

            # ---- stage 1: T_k[(j,r), t] per k_in block, regrouped into tt
            def stage1(k):
                ps_a = psd.tile([128, 512], F32, name=f"s1a{k}", tag=f"s{k % 2}a")
                ps_b = psd.tile([128, 512], F32, name=f"s1b{k}", tag=f"s{k % 2}b")
                for ic in range(4):
                    g = 4 * k + ic
                    mm_pair(
                        ps_a, ps_b,
                        acat_sb[:, g * 128:(g + 1) * 128],
                        xt_sb[:, g * 1024: g * 1024 + 512],
                        xt_sb[:, g * 1024 + 512: (g + 1) * 1024],
                        start=(ic == 0), stop=(ic == 3),
                    )
                ev = evp.tile([128, 1024], BF16, name="ev", tag="ev")
                nc.vector.tensor_copy(ev[:, 0:512], ps_a[:])
                nc.scalar.copy(ev[:, 512:1024], ps_b[:])
                if k < KB - 1:
                    # regroup leg 1: park T_k in DRAM scratch; k=7 skips the
                    # roundtrip (read directly from SBUF, below).
                    nc.sync.dma_start(evd[k], ev[:])
                    # regroup leg 2a, one DMA per k covering every j:
                    # tt[k*16+r, j*1024+t] = evd[k, j*16+r, t]; the (r, j, t)
                    # source order is a plain strided walk of DRAM that
                    # matches the dest's (partition, free) order.
                    nc.sync.dma_start(
                        tt_sb[k * R:(k + 1) * R, :],
                        evd[k].rearrange("(j r) t -> r j t", j=KB),
                    )
                return ev

            def tt_read_hi(j, ev7):
                # regroup leg 2b: k=7 rows come straight from ev7 in SBUF,
                # skipping the DRAM roundtrip on the critical path.
                nc.scalar.dma_start(
                    tt_sb[112:128, j * 1024:(j + 1) * 1024],
                    ev7[j * R:(j + 1) * R, :],
                )

            dense_ps = {}

            def dense_part(oc, lo, hi):
                if lo == 0:
                    dense_ps[oc] = (
                        psd.tile([128, 512], F32, name=f"d0_{oc}", tag=f"d{oc % 2}0"),
                        psd.tile([128, 512], F32, name=f"d1_{oc}", tag=f"d{oc % 2}1"),
                    )
                pd0, pd1 = dense_ps[oc]
                w_t = wslab[oc]
                for ich in range(lo, hi):
                    mm_pair(
                        pd0, pd1,
                        w_t[:, ich * 128:(ich + 1) * 128],
                        xt_sb[:, ich * 1024: ich * 1024 + 512],
                        xt_sb[:, ich * 1024 + 512: (ich + 1) * 1024],
                        start=(ich == 0), stop=False,
                    )
                if hi == NIC:
                    j = oc // 4
                    mm_pair(
                        pd0, pd1,
                        bcat_sb[:, oc * 128:(oc + 1) * 128],
                        tt_sb[:, j * 1024: j * 1024 + 512],
                        tt_sb[:, j * 1024 + 512: (j + 1) * 1024],
                        start=False, stop=True,
                    )
                    o_sb = osbp.tile([128, 1024], BF16, name="osb", tag="osb")
                    nc.vector.tensor_copy(o_sb[:, 0:512], pd0[:])
                    nc.scalar.copy(o_sb[:, 512:1024], pd1[:])
                    nc.scalar.dma_start(
                        outr[oc * 128:(oc + 1) * 128, :], o_sb[:]
                    )
                    del dense_ps[oc]

            # ---- PE program order: stage-1 paced by x-piece arrival;
            # chunk 0/1 partials (which need only early x pieces + w0/w1)
            # fill the DMA-bound startup window.
            stage1(0)
            stage1(1)
            dense_part(0, 0, 8)
            stage1(2)
            dense_part(0, 8, 16)
            stage1(3)
            dense_part(1, 0, 8)
            stage1(4)
            dense_part(0, 16, 24)
            stage1(5)
            dense_part(1, 8, 16)
            stage1(6)
            dense_part(0, 24, 28)
            dense_part(1, 16, 28)
            ev7 = stage1(7)
            for j in range(KB):
                tt_read_hi(j, ev7)
            dense_part(0, 28, 32)
            dense_part(1, 28, 32)
            for oc in range(2, NOC):
                w_dma(oc)
            for oc in range(2, NOC):
                dense_part(oc, 0, 32)

    nc.compile()
    return nc


def _prep(x, W, b, A, B, alpha, r_block):
    x = np.asarray(x, dtype=np.float32)
    W = np.asarray(W, dtype=np.float32)
    b = np.asarray(b, dtype=np.float32)
    A = np.asarray(A, dtype=np.float32)
    B = np.asarray(B, dtype=np.float32)
    scale = float(np.asarray(alpha)) / float(np.asarray(r_block))

    xf = np.ascontiguousarray(x.reshape(-1, D))             # [8192, 4096]
    # wtr[oc*128+p, ich*128+q] = W[oc*128+q, ich*128+p]
    wtr = np.ascontiguousarray(
        W.reshape(32, 128, 32, 128).transpose(0, 3, 2, 1).reshape(D, D)
    ).astype(NPBF16)
    # acat[p, (k*4+ic)*128 + c] = A[k, c//16, c%16, ic*128+p]
    ac = A.transpose(0, 3, 1, 2).reshape(KB, BS, 128)       # [k, i, c]
    acat = np.ascontiguousarray(
        ac.reshape(KB, 4, 128, 128).transpose(2, 0, 1, 3).reshape(128, D)
    ).astype(NPBF16)
    # bcat[k*16+r, j*512+o] = scale * B[k, j, o, r]
    bcat = np.ascontiguousarray(
        (scale * B).transpose(0, 3, 1, 2).reshape(128, D)
    ).astype(NPBF16)
    shards = []
    ntok = xf.shape[0] // N_CORES
    for c in range(N_CORES):
        xs = xf[c * ntok:(c + 1) * ntok]                    # [1024, 4096]
        # xt[p, ich*1024 + t] = xs[t, ich*128 + p]
        xt_host = np.ascontiguousarray(
            xs.reshape(TOK, NIC, 128).transpose(2, 1, 0).reshape(128, NIC * TOK)
        ).astype(NPBF16)
        shards.append(xt_host)
    return shards, wtr, acat, bcat, b, x.shape


def run(x, W, b, A, B, alpha, r_block, trace=False, tmpdir=None):
    shards, wtr, acat, bcat, bb, xshape = _prep(x, W, b, A, B, alpha, r_block)
    if "nc" not in _CACHE:
        _CACHE["nc"] = _build()
    nc = _CACHE["nc"]
    in_maps = [
        {"xt": s, "wtr": wtr, "acat": acat, "bcat": bcat} for s in shards
    ]
    res = run_bass_kernel_spmd(
        nc, in_maps, core_ids=list(range(N_CORES)), trace=trace, tmpdir=tmpdir
    )
    parts = []
    for i in range(N_CORES):
        o = np.asarray(res.results[i]["outr"])              # [4096, 1024] bf16
        parts.append(o.T.astype(np.float32))                # [1024, 4096]
    full = np.concatenate(parts, axis=0)                    # [8192, 4096]
    full = full + bb[None, :]
    return full.reshape(xshape).astype(np.float32), res


def kernel(**inputs):
    out, _ = run(**inputs)
    return out


# revision 28
# speedup vs baseline: 1.0337x; 1.0337x over previous
"""Trainium2 Bass kernel for nn_LocalizedLoraLayer.

Math (full problem):
  out = x @ W.T + b + (alpha/r_block) * delta
  delta[:, :, j*bs:(j+1)*bs] = sum_k  (x_k @ A[k,j].T) @ B[k,j].T
  with x: [4, 2048, 4096], W: [4096, 4096] ([out, in]), A: [8, 8, 16, 512],
  B: [8, 8, 512, 16].

Strategy: data-parallel over tokens (8192 tokens -> 1024/core on 8 cores).
All matmul operands in bf16 (inputs quantized on host; rel err ~4e-3 vs the
2e-2 gate). Output produced transposed [d, tok] in bf16; host untransposes
and upcasts.

Per-core device schedule:
  x resident in SBUF as xt[128(i-local), ich*1024 + t] (8 MiB bf16),
  loaded in 1 MiB pieces split across both HWDGE rings. W streamed once as
  32x 1MiB slabs wsb[128(i-local), ich*128 + o-local].
  Warm-up junk matmuls flip the HAM clock gate to 8/8 during the x load.
  stage 1 (LoRA down-proj): per k_in block, T_k[(j,r), t] accumulated over
    4 i-chunks; evacuated to bf16, parked in DRAM scratch, and read back
    per output block j as tt[(k,r)=128, j*1024 + t] via a strided AP
    (the (k,r,t) source order matches the flat partition order).
  dense: per o-chunk oc (128 outs), psum [128 o, 512 t] x2 token halves
    accumulates 32 dense matmuls (W slab stationary, x moving) plus 1 LoRA
    matmul (bcat stationary, tt moving) = whole layer fused. Each stationary
    feeds both token-half matmuls; the second skips its weight load.
  bias b is added on host during unshard (b is zeros by spec).
"""

import numpy as np
import ml_dtypes

import concourse.bass as bass
import concourse.mybir as mybir
import concourse.tile as tile
from concourse import bacc
from concourse.bass_utils import run_bass_kernel_spmd

N_CORES = 8
TOK = 1024          # tokens per core
D = 4096            # model dim
KB = 8              # number of blocks (K)
BS = 512            # block size
R = 16              # lora rank
NIC = D // 128      # 32 i-chunks
NOC = D // 128      # 32 o-chunks of 128

F32 = mybir.dt.float32
BF16 = mybir.dt.bfloat16
NPBF16 = ml_dtypes.bfloat16

_CACHE = {}


def _build():
    nc = bacc.Bacc(None, target_bir_lowering=False)

    xt = nc.dram_tensor("xt", [128, NIC * TOK], BF16, kind="ExternalInput")
    wtr = nc.dram_tensor("wtr", [D, D], BF16, kind="ExternalInput")
    acat = nc.dram_tensor("acat", [128, D], BF16, kind="ExternalInput")
    bcat = nc.dram_tensor("bcat", [128, D], BF16, kind="ExternalInput")
    outr = nc.dram_tensor("outr", [D, TOK], BF16, kind="ExternalOutput")

    def mm_pair(p0, p1, lhsT, r0, r1, start, stop):
        # two matmuls sharing one stationary operand
        nc.tensor.matmul(p0[:], lhsT, r0, start=start, stop=stop)
        nc.tensor.matmul(p1[:], lhsT, r1, start=start, stop=stop)

    with tile.TileContext(nc) as tc:
        with (
            tc.tile_pool(name="res", bufs=1) as res,
            tc.tile_pool(name="wts", bufs=5) as wts,
            tc.tile_pool(name="evp", bufs=4) as evp,
            tc.tile_pool(name="osb", bufs=4) as osbp,
            tc.tile_pool(name="psd", bufs=1, space="PSUM") as psd,
            tc.tile_pool(name="dramp", bufs=1, space="DRAM") as dramp,
        ):
            acat_sb = res.tile([128, D], BF16)
            xt_sb = res.tile([128, NIC * TOK], BF16)
            bcat_sb = res.tile([128, D], BF16)
            tt_sb = res.tile([128, KB * TOK], BF16)
            scr_sb = res.tile([128, 512], BF16)
            evd = dramp.tile([KB, 128, TOK], BF16)

            wslab = {}

            def w_dma(oc):
                w_t = wts.tile([128, D], BF16, name=f"w{oc}", tag="w")
                nc.sync.dma_start(
                    w_t[:], wtr[oc * 128:(oc + 1) * 128, :]
                )
                wslab[oc] = w_t

            def x_dma(p):
                nc.sync.dma_start(
                    xt_sb[:, p * 4096:(p + 1) * 4096],
                    xt[:, p * 4096:(p + 1) * 4096],
                )

            def x_dma_s(p):
                nc.scalar.dma_start(
                    xt_sb[:, p * 4096:(p + 1) * 4096],
                    xt[:, p * 4096:(p + 1) * 4096],
                )

            # sync: acat head, x0a, x1, w0, x3, x5, x7, [ev/tt regroup], w2..
            # scalar: acat tail, x0b, x2, x4, x6, w1, bcat, [evacs/outs]
            nc.sync.dma_start(acat_sb[:, 0:1024], acat[:, 0:1024])
            nc.sync.dma_start(xt_sb[:, 0:2048], xt[:, 0:2048])
            nc.scalar.dma_start(acat_sb[:, 1024:4096], acat[:, 1024:4096])
            nc.scalar.dma_start(xt_sb[:, 2048:4096], xt[:, 2048:4096])
            x_dma(1)
            x_dma_s(2)
            w_dma(0)
            x_dma(3)
            x_dma_s(4)
            x_dma(5)
            x_dma_s(6)
            x_dma(7)
            w1_t = wts.tile([128, D], BF16, name="w1", tag="w")
            nc.scalar.dma_start(w1_t[:], wtr[128:256, :])
            wslab[1] = w1_t
            nc.scalar.dma_start(bcat_sb[:], bcat[:])

            # ---- PE warm-up: ~12 junk matmuls on scratch data flip the
            # HAM clock gate to 8/8 during the initial x load, so stage-1
            # runs at full clock. Output goes to a dead PSUM bank.
            nc.vector.memset(scr_sb[:], 0.0)
            ps_w = psd.tile([128, 512], F32, name="warm", tag="d10")
            for i in range(12):
                nc.tensor.matmul(
                    ps_w[:], scr_sb[:, 0:128], scr_sb[:],
                    start=(i == 0), stop=(i == 11),
                )

            # ---- stage 1: T_k[(j,r), t] per k_in block, regrouped into tt
            def stage1(k):
                ps_a = psd.tile([128, 512], F32, name=f"s1a{k}", tag=f"s{k % 2}a")
                ps_b = psd.tile([128, 512], F32, name=f"s1b{k}", tag=f"s{k % 2}b")
                for ic in range(4):
                    g = 4 * k + ic
                    mm_pair(
                        ps_a, ps_b,
                        acat_sb[:, g * 128:(g + 1) * 128],
                        xt_sb[:, g * 1024: g * 1024 + 512],
                        xt_sb[:, g * 1024 + 512: (g + 1) * 1024],
                        start=(ic == 0), stop=(ic == 3),
                    )
                ev = evp.tile([128, 1024], BF16, name="ev", tag="ev")
                nc.vector.tensor_copy(ev[:, 0:512], ps_a[:])
                nc.scalar.copy(ev[:, 512:1024], ps_b[:])
                if k < KB - 1:
                    # regroup leg 1: park T_k in DRAM scratch; k=7 skips the
                    # roundtrip (read directly from SBUF, below).
                    nc.sync.dma_start(evd[k], ev[:])
                return ev

            def tt_read_lo(j):
                # regroup leg 2a: tt[k*16+r, j*1024+t] = evd[k, j*16+r, t]
                # for k=0..6; the strided 3-D source iterates (k, r, t)
                # which matches the flat (partition, t) dest order.
                nc.sync.dma_start(
                    tt_sb[0:112, j * 1024:(j + 1) * 1024],
                    evd[0:KB - 1, j * R:(j + 1) * R, :],
                )

            def tt_read_hi(j, ev7):
                # regroup leg 2b: k=7 rows come straight from ev7 in SBUF,
                # skipping the DRAM roundtrip on the critical path.
                nc.scalar.dma_start(
                    tt_sb[112:128, j * 1024:(j + 1) * 1024],
                    ev7[j * R:(j + 1) * R, :],
                )

            dense_ps = {}

            def dense_part(oc, lo, hi):
                if lo == 0:
                    dense_ps[oc] = (
                        psd.tile([128, 512], F32, name=f"d0_{oc}", tag=f"d{oc % 2}0"),
                        psd.tile([128, 512], F32, name=f"d1_{oc}", tag=f"d{oc % 2}1"),
                    )
                pd0, pd1 = dense_ps[oc]
                w_t = wslab[oc]
                for ich in range(lo, hi):
                    mm_pair(
                        pd0, pd1,
                        w_t[:, ich * 128:(ich + 1) * 128],
                        xt_sb[:, ich * 1024: ich * 1024 + 512],
                        xt_sb[:, ich * 1024 + 512: (ich + 1) * 1024],
                        start=(ich == 0), stop=False,
                    )
                if hi == NIC:
                    j = oc // 4
                    mm_pair(
                        pd0, pd1,
                        bcat_sb[:, oc * 128:(oc + 1) * 128],
                        tt_sb[:, j * 1024: j * 1024 + 512],
                        tt_sb[:, j * 1024 + 512: (j + 1) * 1024],
                        start=False, stop=True,
                    )
                    o_sb = osbp.tile([128, 1024], BF16, name="osb", tag="osb")
                    nc.vector.tensor_copy(o_sb[:, 0:512], pd0[:])
                    nc.scalar.copy(o_sb[:, 512:1024], pd1[:])
                    nc.scalar.dma_start(
                        outr[oc * 128:(oc + 1) * 128, :], o_sb[:]
                    )
                    del dense_ps[oc]

            # ---- PE program order: stage-1 paced by x-piece arrival;
            # chunk 0/1 partials (which need only early x pieces + w0/w1)
            # fill the DMA-bound startup window.
            stage1(0)
            stage1(1)
            dense_part(0, 0, 8)
            stage1(2)
            dense_part(0, 8, 16)
            stage1(3)
            dense_part(1, 0, 8)
            stage1(4)
            dense_part(0, 16, 24)
            stage1(5)
            dense_part(1, 8, 16)
            stage1(6)
            for j in range(KB):
                tt_read_lo(j)
            dense_part(0, 24, 28)
            dense_part(1, 16, 28)
            ev7 = stage1(7)
            for j in range(KB):
                tt_read_hi(j, ev7)
            dense_part(0, 28, 32)
            dense_part(1, 28, 32)
            for oc in range(2, NOC):
                w_dma(oc)
            for oc in range(2, NOC):
                dense_part(oc, 0, 32)

    nc.compile()
    return nc


def _prep(x, W, b, A, B, alpha, r_block):
    x = np.asarray(x, dtype=np.float32)
    W = np.asarray(W, dtype=np.float32)
    b = np.asarray(b, dtype=np.float32)
    A = np.asarray(A, dtype=np.float32)
    B = np.asarray(B, dtype=np.float32)
    scale = float(np.asarray(alpha)) / float(np.asarray(r_block))

    xf = np.ascontiguousarray(x.reshape(-1, D))             # [8192, 4096]
    # wtr[oc*128+p, ich*128+q] = W[oc*128+q, ich*128+p]
    wtr = np.ascontiguousarray(
        W.reshape(32, 128, 32, 128).transpose(0, 3, 2, 1).reshape(D, D)
    ).astype(NPBF16)
    # acat[p, (k*4+ic)*128 + c] = A[k, c//16, c%16, ic*128+p]
    ac = A.transpose(0, 3, 1, 2).reshape(KB, BS, 128)       # [k, i, c]
    acat = np.ascontiguousarray(
        ac.reshape(KB, 4, 128, 128).transpose(2, 0, 1, 3).reshape(128, D)
    ).astype(NPBF16)
    # bcat[k*16+r, j*512+o] = scale * B[k, j, o, r]
    bcat = np.ascontiguousarray(
        (scale * B).transpose(0, 3, 1, 2).reshape(128, D)
    ).astype(NPBF16)
    shards = []
    ntok = xf.shape[0] // N_CORES
    for c in range(N_CORES):
        xs = xf[c * ntok:(c + 1) * ntok]                    # [1024, 4096]
        # xt[p, ich*1024 + t] = xs[t, ich*128 + p]
        xt_host = np.ascontiguousarray(
            xs.reshape(TOK, NIC, 128).transpose(2, 1, 0).reshape(128, NIC * TOK)
        ).astype(NPBF16)
        shards.append(xt_host)
    return shards, wtr, acat, bcat, b, x.shape


def run(x, W, b, A, B, alpha, r_block, trace=False, tmpdir=None):
    shards, wtr, acat, bcat, bb, xshape = _prep(x, W, b, A, B, alpha, r_block)
    if "nc" not in _CACHE:
        _CACHE["nc"] = _build()
    nc = _CACHE["nc"]
    in_maps = [
        {"xt": s, "wtr": wtr, "acat": acat, "bcat": bcat} for s in shards
    ]
    res = run_bass_kernel_spmd(
        nc, in_maps, core_ids=list(range(N_CORES)), trace=trace, tmpdir=tmpdir
    )
    parts = []
    for i in range(N_CORES):
        o = np.asarray(res.results[i]["outr"])              # [4096, 1024] bf16
        parts.append(o.T.astype(np.float32))                # [1024, 4096]
    full = np.concatenate(parts, axis=0)                    # [8192, 4096]
    full = full + bb[None, :]
    return full.reshape(xshape).astype(np.float32), res


def kernel(**inputs):
    out, _ = run(**inputs)
    return out


# revision 30
# speedup vs baseline: 1.0402x; 1.0063x over previous
"""Trainium2 Bass kernel for nn_LocalizedLoraLayer.

Math (full problem):
  out = x @ W.T + b + (alpha/r_block) * delta
  delta[:, :, j*bs:(j+1)*bs] = sum_k  (x_k @ A[k,j].T) @ B[k,j].T
  with x: [4, 2048, 4096], W: [4096, 4096] ([out, in]), A: [8, 8, 16, 512],
  B: [8, 8, 512, 16].

Strategy: data-parallel over tokens (8192 tokens -> 1024/core on 8 cores).
All matmul operands in bf16 (inputs quantized on host; rel err ~4e-3 vs the
2e-2 gate). Output produced transposed [d, tok] in bf16; host untransposes
and upcasts.

Per-core device schedule:
  x resident in SBUF as xt[128(i-local), ich*1024 + t] (8 MiB bf16),
  loaded in 1 MiB pieces split across both HWDGE rings. W streamed once as
  32x 1MiB slabs wsb[128(i-local), ich*128 + o-local].
  Warm-up junk matmuls flip the HAM clock gate to 8/8 during the x load.
  stage 1 (LoRA down-proj): per k_in block, T_k[(j,r), t] accumulated over
    4 i-chunks; evacuated to bf16, parked in DRAM scratch, and read back
    per output block j as tt[(k,r)=128, j*1024 + t] via a strided AP
    (the (k,r,t) source order matches the flat partition order).
  dense: per o-chunk oc (128 outs), psum [128 o, 512 t] x2 token halves
    accumulates 32 dense matmuls (W slab stationary, x moving) plus 1 LoRA
    matmul (bcat stationary, tt moving) = whole layer fused. Each stationary
    feeds both token-half matmuls; the second skips its weight load.
  bias b is added on host during unshard (b is zeros by spec).
"""

import numpy as np
import ml_dtypes

import concourse.bass as bass
import concourse.mybir as mybir
import concourse.tile as tile
from concourse import bacc
from concourse.bass_utils import run_bass_kernel_spmd

N_CORES = 8
TOK = 1024          # tokens per core
D = 4096            # model dim
KB = 8              # number of blocks (K)
BS = 512            # block size
R = 16              # lora rank
NIC = D // 128      # 32 i-chunks
NOC = D // 128      # 32 o-chunks of 128

F32 = mybir.dt.float32
BF16 = mybir.dt.bfloat16
NPBF16 = ml_dtypes.bfloat16

_CACHE = {}


def _build():
    nc = bacc.Bacc(None, target_bir_lowering=False)

    xt = nc.dram_tensor("xt", [128, NIC * TOK], BF16, kind="ExternalInput")
    wtr = nc.dram_tensor("wtr", [D, D], BF16, kind="ExternalInput")
    acat = nc.dram_tensor("acat", [128, D], BF16, kind="ExternalInput")
    bcat = nc.dram_tensor("bcat", [128, D], BF16, kind="ExternalInput")
    outr = nc.dram_tensor("outr", [D, TOK], BF16, kind="ExternalOutput")

    def mm_pair(p0, p1, lhsT, r0, r1, start, stop):
        # two matmuls sharing one stationary operand
        nc.tensor.matmul(p0[:], lhsT, r0, start=start, stop=stop)
        nc.tensor.matmul(p1[:], lhsT, r1, start=start, stop=stop)

    with tile.TileContext(nc) as tc:
        with (
            tc.tile_pool(name="res", bufs=1) as res,
            tc.tile_pool(name="wts", bufs=5) as wts,
            tc.tile_pool(name="evp", bufs=4) as evp,
            tc.tile_pool(name="osb", bufs=4) as osbp,
            tc.tile_pool(name="psd", bufs=1, space="PSUM") as psd,
            tc.tile_pool(name="dramp", bufs=1, space="DRAM") as dramp,
        ):
            acat_sb = res.tile([128, D], BF16)
            xt_sb = res.tile([128, NIC * TOK], BF16)
            bcat_sb = res.tile([128, D], BF16)
            tt_sb = res.tile([128, KB * TOK], BF16)
            scr_sb = res.tile([128, 512], BF16)
            evd = dramp.tile([KB, 128, TOK], BF16)

            wslab = {}

            def w_dma(oc):
                w_t = wts.tile([128, D], BF16, name=f"w{oc}", tag="w")
                nc.sync.dma_start(
                    w_t[:], wtr[oc * 128:(oc + 1) * 128, :]
                )
                wslab[oc] = w_t

            def x_dma(p):
                nc.sync.dma_start(
                    xt_sb[:, p * 4096:(p + 1) * 4096],
                    xt[:, p * 4096:(p + 1) * 4096],
                )

            def x_dma_s(p):
                nc.scalar.dma_start(
                    xt_sb[:, p * 4096:(p + 1) * 4096],
                    xt[:, p * 4096:(p + 1) * 4096],
                )

            # sync: acat head, x0a, x1, w0, x3, x5, x7, [ev/tt regroup], w2..
            # scalar: acat tail, x0b, x2, x4, x6, w1, bcat, [evacs/outs]
            nc.sync.dma_start(acat_sb[:, 0:1024], acat[:, 0:1024])
            nc.sync.dma_start(xt_sb[:, 0:2048], xt[:, 0:2048])
            nc.scalar.dma_start(acat_sb[:, 1024:4096], acat[:, 1024:4096])
            nc.scalar.dma_start(xt_sb[:, 2048:4096], xt[:, 2048:4096])
            x_dma(1)
            x_dma_s(2)
            w_dma(0)
            x_dma(3)
            x_dma_s(4)
            x_dma(5)
            x_dma_s(6)
            x_dma(7)
            nc.sync.dma_start(bcat_sb[:], bcat[:])
            w1_t = wts.tile([128, D], BF16, name="w1", tag="w")
            nc.scalar.dma_start(w1_t[:], wtr[128:256, :])
            wslab[1] = w1_t

            # ---- PE warm-up: ~12 junk matmuls on scratch data flip the
            # HAM clock gate to 8/8 during the initial x load, so stage-1
            # runs at full clock. Output goes to a dead PSUM bank.
            nc.vector.memset(scr_sb[:], 0.0)
            ps_w = psd.tile([128, 512], F32, name="warm", tag="d10")
            for i in range(12):
                nc.tensor.matmul(
                    ps_w[:], scr_sb[:, 0:128], scr_sb[:],
                    start=(i == 0), stop=(i == 11),
                )

            # ---- stage 1: T_k[(j,r), t] per k_in block, regrouped into tt
            def stage1(k):
                ps_a = psd.tile([128, 512], F32, name=f"s1a{k}", tag=f"s{k % 2}a")
                ps_b = psd.tile([128, 512], F32, name=f"s1b{k}", tag=f"s{k % 2}b")
                for ic in range(4):
                    g = 4 * k + ic
                    mm_pair(
                        ps_a, ps_b,
                        acat_sb[:, g * 128:(g + 1) * 128],
                        xt_sb[:, g * 1024: g * 1024 + 512],
                        xt_sb[:, g * 1024 + 512: (g + 1) * 1024],
                        start=(ic == 0), stop=(ic == 3),
                    )
                ev = evp.tile([128, 1024], BF16, name="ev", tag="ev")
                nc.vector.tensor_copy(ev[:, 0:512], ps_a[:])
                nc.scalar.copy(ev[:, 512:1024], ps_b[:])
                if k < KB - 1:
                    # regroup leg 1: park T_k in DRAM scratch; k=7 skips the
                    # roundtrip (read directly from SBUF, below).
                    nc.sync.dma_start(evd[k], ev[:])
                return ev

            def tt_read_lo(j):
                # regroup leg 2a: tt[k*16+r, j*1024+t] = evd[k, j*16+r, t]
                # for k=0..6; the strided 3-D source iterates (k, r, t)
                # which matches the flat (partition, t) dest order.
                nc.sync.dma_start(
                    tt_sb[0:112, j * 1024:(j + 1) * 1024],
                    evd[0:KB - 1, j * R:(j + 1) * R, :],
                )

            def tt_read_hi(j, ev7):
                # regroup leg 2b: k=7 rows come straight from ev7 in SBUF,
                # skipping the DRAM roundtrip on the critical path.
                nc.scalar.dma_start(
                    tt_sb[112:128, j * 1024:(j + 1) * 1024],
                    ev7[j * R:(j + 1) * R, :],
                )

            dense_ps = {}

            def dense_part(oc, lo, hi):
                if lo == 0:
                    dense_ps[oc] = (
                        psd.tile([128, 512], F32, name=f"d0_{oc}", tag=f"d{oc % 2}0"),
                        psd.tile([128, 512], F32, name=f"d1_{oc}", tag=f"d{oc % 2}1"),
                    )
                pd0, pd1 = dense_ps[oc]
                w_t = wslab[oc]
                for ich in range(lo, hi):
                    mm_pair(
                        pd0, pd1,
                        w_t[:, ich * 128:(ich + 1) * 128],
                        xt_sb[:, ich * 1024: ich * 1024 + 512],
                        xt_sb[:, ich * 1024 + 512: (ich + 1) * 1024],
                        start=(ich == 0), stop=False,
                    )
            def dense_close(oc):
                # LoRA fold-in (33rd accumulating matmul) + evacuation
                pd0, pd1 = dense_ps[oc]
                j = oc // 4
                mm_pair(
                    pd0, pd1,
                    bcat_sb[:, oc * 128:(oc + 1) * 128],
                    tt_sb[:, j * 1024: j * 1024 + 512],
                    tt_sb[:, j * 1024 + 512: (j + 1) * 1024],
                    start=False, stop=True,
                )
                o_sb = osbp.tile([128, 1024], BF16, name="osb", tag="osb")
                nc.vector.tensor_copy(o_sb[:, 0:512], pd0[:])
                nc.scalar.copy(o_sb[:, 512:1024], pd1[:])
                nc.scalar.dma_start(
                    outr[oc * 128:(oc + 1) * 128, :], o_sb[:]
                )
                del dense_ps[oc]

            # ---- PE program order: stage-1 paced by x-piece arrival;
            # chunk 0/1 partials (which need only early x pieces + w0/w1)
            # fill the DMA-bound startup window.
            stage1(0)
            stage1(1)
            dense_part(0, 0, 8)
            stage1(2)
            dense_part(0, 8, 16)
            stage1(3)
            dense_part(1, 0, 8)
            stage1(4)
            dense_part(0, 16, 24)
            stage1(5)
            dense_part(1, 8, 16)
            stage1(6)
            for j in range(KB):
                tt_read_lo(j)
            dense_part(0, 24, 28)
            dense_part(1, 16, 24)
            ev7 = stage1(7)
            for j in range(KB):
                tt_read_hi(j, ev7)
            dense_part(1, 24, 28)
            dense_part(0, 28, 32)
            dense_part(1, 28, 32)
            dense_close(0)
            dense_close(1)
            for oc in range(2, NOC):
                w_dma(oc)
            for oc in range(2, NOC):
                dense_part(oc, 0, 32)
                dense_close(oc)

    nc.compile()
    return nc


def _prep(x, W, b, A, B, alpha, r_block):
    x = np.asarray(x, dtype=np.float32)
    W = np.asarray(W, dtype=np.float32)
    b = np.asarray(b, dtype=np.float32)
    A = np.asarray(A, dtype=np.float32)
    B = np.asarray(B, dtype=np.float32)
    scale = float(np.asarray(alpha)) / float(np.asarray(r_block))

    xf = np.ascontiguousarray(x.reshape(-1, D))             # [8192, 4096]
    # wtr[oc*128+p, ich*128+q] = W[oc*128+q, ich*128+p]
    wtr = np.ascontiguousarray(
        W.reshape(32, 128, 32, 128).transpose(0, 3, 2, 1).reshape(D, D)
    ).astype(NPBF16)
    # acat[p, (k*4+ic)*128 + c] = A[k, c//16, c%16, ic*128+p]
    ac = A.transpose(0, 3, 1, 2).reshape(KB, BS, 128)       # [k, i, c]
    acat = np.ascontiguousarray(
        ac.reshape(KB, 4, 128, 128).transpose(2, 0, 1, 3).reshape(128, D)
    ).astype(NPBF16)
    # bcat[k*16+r, j*512+o] = scale * B[k, j, o, r]
    bcat = np.ascontiguousarray(
        (scale * B).transpose(0, 3, 1, 2).reshape(128, D)
    ).astype(NPBF16)
    shards = []
    ntok = xf.shape[0] // N_CORES
    for c in range(N_CORES):
        xs = xf[c * ntok:(c + 1) * ntok]                    # [1024, 4096]
        # xt[p, ich*1024 + t] = xs[t, ich*128 + p]
        xt_host = np.ascontiguousarray(
            xs.reshape(TOK, NIC, 128).transpose(2, 1, 0).reshape(128, NIC * TOK)
        ).astype(NPBF16)
        shards.append(xt_host)
    return shards, wtr, acat, bcat, b, x.shape


def run(x, W, b, A, B, alpha, r_block, trace=False, tmpdir=None):
    shards, wtr, acat, bcat, bb, xshape = _prep(x, W, b, A, B, alpha, r_block)
    if "nc" not in _CACHE:
        _CACHE["nc"] = _build()
    nc = _CACHE["nc"]
    in_maps = [
        {"xt": s, "wtr": wtr, "acat": acat, "bcat": bcat} for s in shards
    ]
    res = run_bass_kernel_spmd(
        nc, in_maps, core_ids=list(range(N_CORES)), trace=trace, tmpdir=tmpdir
    )
    parts = []
    for i in range(N_CORES):
        o = np.asarray(res.results[i]["outr"])              # [4096, 1024] bf16
        parts.append(o.T.astype(np.float32))                # [1024, 4096]
    full = np.concatenate(parts, axis=0)                    # [8192, 4096]
    full = full + bb[None, :]
    return full.reshape(xshape).astype(np.float32), res


def kernel(**inputs):
    out, _ = run(**inputs)
    return out
